# revision 15
# baseline (speedup 1.0000x reference)
"""Trainium2 Bass kernel for nn_ChromaticResonance (v2).

Per batch row, complex wave w of dim D=512, 7 depths of:
  y  = w@(C+H1) [+ w for d>0, folded as +I into the matrix]
       + 0.25*|w@H2|^2                       (real only)
       + (1/9)*|w@H3|^2 * (w@H3)
       + 0.04*(w@H5)^5 * |w@H5|^-4.8
  nl = tanh(y*scale + bias)   (componentwise re/im)
  w' = exp(-damping*d) * nl
  out = sum_d w_d * w'_d

v2 strategy (vs v1 which ran fp32r matmuls and assembled acc on DVE):
  - All matmuls bf16 (1 cyc/row like fp32r, but LDWEIGHTS is FWL-fast and
    fully hidden, and SBUF/DMA halve). Whole-pipeline bf16 rel err ~5e-3
    (validated vs numpy model), budget is 2e-2.
  - acc lives in PSUM (the W1 bank): the h2/h3/h5 terms are injected with
    identity matmuls (bf16 pass-through is exact) instead of DVE adds.
  - Per (depth,m) unit: wave A = [H5, H3] matmuls, elementwise chain,
    wave B = [W1->psB(bufs=2), H2]; the acc finalization (identities +
    tanh + ch_next + out accum) is SOFTWARE-PIPELINED one unit behind so
    the PE never waits on the ~8us elementwise dependency chain.
  - Chunks processed in pairs, with the second chunk's units emitted
    between depths of the first: the depth barrier (ch_next of all 4
    m-tiles) hides behind ~28us of the other chunk's work.
  - h5 via packed complex ops: z^2/z^4 squares, z^5 = z^4*(t5*sqrt2*z)
    with dup2 (stride-0) and swapped-half APs so DVE does full-width
    2-byte 2x ops; (r^2)^-2.4 via the bitcast-log2 + quadratic mantissa
    fix + single table Exp (keeps the one ACT table set: no reloads).
"""

import numpy as np
import ml_dtypes

import concourse.bass as bass
import concourse.mybir as mybir
import concourse.tile as tile
from concourse import bass_utils
from concourse.bacc import Bacc

F32 = mybir.dt.float32
BF16 = mybir.dt.bfloat16
I32 = mybir.dt.int32
AF = mybir.ActivationFunctionType
OP = mybir.AluOpType

B, D, DEPTH = 32768, 512, 7
N_CORES = 8
BS = B // N_CORES          # batch rows per core
NB = 512                   # batch columns per chunk
KT = D // 128              # 4 partition tiles of the D dim
# fast-log2 (bitcast): log2(x) ~ 2^-23*i - 127 + 0.043, the sigma(f)
# mantissa-bump replaced by its mean (t5 err +-7%, h5 is 3% of acc -> ~0.03%
# end-to-end, validated in numpy model).
EXP_SCALE = float(-2.4 * np.log(2.0) * 2.0 ** -23)
EXP_BIAS = float(127 * 2.4 * np.log(2.0) + np.log(0.04)
                 - 2.4 * np.log(2.0) * 0.0430)
H3_FOLD = float(9.0 ** (-1.0 / 3.0))   # |hw3'|^2*hw3' = |hw3|^2*hw3/9


def _dup2(t, n):
    """Broadcast a [128, n] AP to [128, 2, n] (each column read twice)."""
    ap = t.ap
    return bass.AP(tensor=t.tensor, offset=t.offset, ap=[ap[0], [0, 2], ap[1]])


def _swap2(t, nb):
    """View a [128, 2*nb] packed tile as [128, 2, nb] with halves swapped."""
    u = t[:, nb:2 * nb]
    ap = u.ap
    return bass.AP(tensor=u.tensor, offset=u.offset, ap=[ap[0], [-nb, 2], ap[1]])


def _as3(t):
    return t.rearrange("p (two n) -> p two n", two=2)


def build_program(n_chunks=BS // NB, nb=NB):
    nc = Bacc()
    bcols = n_chunks * nb

    wre = nc.dram_tensor("wre", [D, bcols], BF16, kind="ExternalInput")
    wim = nc.dram_tensor("wim", [D, bcols], BF16, kind="ExternalInput")
    wmat = nc.dram_tensor("wmat", [5, D, D], BF16, kind="ExternalInput")
    idm = nc.dram_tensor("idm", [3, 128, 128], BF16, kind="ExternalInput")
    consts = nc.dram_tensor("consts", [D, 16], F32, kind="ExternalInput")
    ore = nc.dram_tensor("ore", [D, bcols], BF16, kind="ExternalOutput")
    oim = nc.dram_tensor("oim", [D, bcols], BF16, kind="ExternalOutput")

    H = slice(0, nb)       # real half of a packed tile
    I = slice(nb, 2 * nb)  # imag half

    with tile.TileContext(nc) as tc:
        with (
            tc.tile_pool(name="wpool", bufs=1) as wpool,
            tc.tile_pool(name="chpool", bufs=2) as chpool,
            tc.tile_pool(name="opool", bufs=1) as opool,
            tc.tile_pool(name="ppool", bufs=1, space="PSUM") as ppool,
            tc.tile_pool(name="sf", bufs=2) as sf,     # f32 scratch
            tc.tile_pool(name="sh", bufs=2) as sh,     # bf16 scratch
        ):
            # ---- load weights + constants (once) ----
            wt = []
            for mi in range(5):
                w = wpool.tile([128, KT, D], BF16, name=f"wt{mi}", tag=f"wt{mi}")
                for k in range(KT):
                    nc.sync.dma_start(out=w[:, k, :], in_=wmat[mi, k * 128:(k + 1) * 128, :])
                wt.append(w)
            # identity stationaries: I, +4I, -4I (the scaled ones absorb the
            # complex-square factor 4 on the q24 injection)
            idt = wpool.tile([128, 3, 128], BF16, name="idt", tag="idt")
            for j in range(3):
                nc.sync.dma_start(out=idt[:, j, :], in_=idm[j, :, :])
            cons = []
            for m in range(KT):
                c = wpool.tile([128, 16], F32, name=f"cons{m}", tag=f"cons{m}")
                nc.sync.dma_start(out=c, in_=consts[m * 128:(m + 1) * 128, :])
                cons.append(c)

            def emit_identities(u):
                mm = nc.tensor.matmul
                psB = u["psB"]
                ID, P4, N4 = idt[:, 0, :], idt[:, 1, :], idt[:, 2, :]
                mm(psB[:, H], ID, u["ht3"][:, H], start=False, stop=False)
                mm(psB[:, I], ID, u["ht3"][:, I], start=False, stop=False)
                mm(psB[:, H], ID, u["q13"][:, H], start=False, stop=False)
                mm(psB[:, I], ID, u["q13"][:, I], start=False, stop=False)
                mm(psB[:, H], N4, u["q24"][:, H], start=False, stop=False)
                mm(psB[:, I], P4, u["q24"][:, I], start=False, stop=True)
                mm(psB[:, H], ID, u["r2a"][:, :], start=False, stop=True)

            def emit_tanh(u):
                m = u["m"]
                nl = sh.tile([128, 2 * nb], BF16, name="nl", tag="nl")
                nc.scalar.activation(nl, u["psB"][:, :], AF.Tanh,
                                     scale=cons[m][:, 14:15], bias=cons[m][:, 15:16])
                u["nl"] = nl

            def emit_fin_dve(u):
                d, c, m, nl = u["d"], u["c"], u["m"], u["nl"]
                if d < DEPTH - 1:
                    nc.vector.tensor_scalar_mul(u["nxt"][m][:, :], nl,
                                                cons[m][:, d:d + 1])
                out_t = u["out_t"]
                if d == 0:
                    nc.vector.tensor_scalar_mul(out_t[:, :], nl, cons[m][:, 7:8])
                else:
                    nc.vector.scalar_tensor_tensor(out_t[:, :], nl,
                                                   cons[m][:, 7 + d:8 + d],
                                                   out_t[:, :], op0=OP.mult, op1=OP.add)
                if d == DEPTH - 1:
                    c0 = c * nb
                    nc.sync.dma_start(out=ore[m * 128:(m + 1) * 128, c0:c0 + nb],
                                      in_=out_t[:, H])
                    nc.sync.dma_start(out=oim[m * 128:(m + 1) * 128, c0:c0 + nb],
                                      in_=out_t[:, I])

            def emit_step(d, c, m, ch, nxt, out_t, prev):
                """Emit unit (d,c,m); finalize of `prev` is interleaved at
                dependency-optimal points so no engine queue-head stalls."""
                msl = slice(m * 128, (m + 1) * 128)
                w1 = wt[0] if d == 0 else wt[1]
                mm = nc.tensor.matmul
                # ---- PE: wave A (H5, H3), then prev's acc injections ----
                ps5 = ppool.tile([128, 2 * nb], F32, name="ps5", tag="ps5")
                psA = ppool.tile([128, 2 * nb], F32, name="psA", tag="psA")
                for k in range(KT):
                    for hs in (H, I):
                        mm(ps5[:, hs], wt[4][:, k, msl], ch[k][:, hs],
                           start=(k == 0), stop=(k == KT - 1))
                for k in range(KT):
                    for hs in (H, I):
                        mm(psA[:, hs], wt[3][:, k, msl], ch[k][:, hs],
                           start=(k == 0), stop=(k == KT - 1))
                if prev is not None:
                    emit_identities(prev)
                # ---- Act: evacuate PSUM, then prev's tanh ----
                sq5 = sf.tile([128, 2 * nb], F32, name="sq5", tag="sq5")
                nc.scalar.activation(sq5, ps5[:, :], AF.Square)
                d5 = sh.tile([128, 2 * nb], BF16, name="d5", tag="d5")
                nc.scalar.copy(d5, ps5[:, :])
                cp3 = sh.tile([128, 2 * nb], BF16, name="cp3", tag="cp3")
                nc.scalar.copy(cp3, psA[:, :])
                if prev is not None:
                    emit_tanh(prev)
                # ---- DVE: log-trick start + d5-products; Pool: u,v ----
                r2d = sf.tile([128, nb], F32, name="r2d", tag="r2d")
                nc.vector.tensor_tensor(r2d, sq5[:, H], sq5[:, I], op=OP.add)
                i_f = sf.tile([128, nb], F32, name="i_f", tag="i_f")
                nc.vector.tensor_scalar(i_f, r2d[:, :].bitcast(I32), 0,
                                        None, op0=OP.add)
                uv = sh.tile([128, 2 * nb], BF16, name="uv", tag="uv")
                nc.gpsimd.tensor_tensor(uv[:, H], d5[:, H], d5[:, I], op=OP.subtract)
                nc.gpsimd.tensor_tensor(uv[:, I], d5[:, H], d5[:, I], op=OP.add)
                c2i = sh.tile([128, nb], BF16, name="c2i", tag="c2i")
                nc.vector.tensor_tensor(c2i, d5[:, H], d5[:, I], op=OP.mult)
                c2r = sh.tile([128, nb], BF16, name="c2r", tag="c2r")
                nc.vector.tensor_tensor(c2r, uv[:, H], uv[:, I], op=OP.mult)
                c4i = sh.tile([128, nb], BF16, name="c4i", tag="c4i")
                nc.vector.tensor_tensor(c4i, c2r, c2i, op=OP.mult)
                sq3 = sh.tile([128, 2 * nb], BF16, name="sq3", tag="sq3")
                nc.vector.tensor_tensor(sq3, cp3[:, :], cp3[:, :], op=OP.mult)
                # ---- prev's DVE finalize lands here (tanh just done) ----
                if prev is not None:
                    emit_fin_dve(prev)
                # ---- Act: z^4 squares + Exp ----
                sqz = sh.tile([128, 2 * nb], BF16, name="sqz", tag="sqz")
                nc.scalar.activation(sqz[:, H], c2r, AF.Square)
                nc.scalar.activation(sqz[:, I], c2i, AF.Square, scale=2.0)
                t5 = sh.tile([128, nb], BF16, name="t5", tag="t5")
                nc.scalar.activation(t5, i_f, AF.Exp, scale=EXP_SCALE,
                                     bias=cons[m][:, 6:7])
                # ---- DVE: h5 tail; Pool: r2b ----
                c4r = sh.tile([128, nb], BF16, name="c4r", tag="c4r")
                nc.vector.tensor_tensor(c4r, sqz[:, H], sqz[:, I], op=OP.subtract)
                tz = sh.tile([128, 2 * nb], BF16, name="tz", tag="tz")
                nc.vector.tensor_tensor(_as3(tz), _dup2(t5, nb), _as3(d5), op=OP.mult)
                q13 = sh.tile([128, 2 * nb], BF16, name="q13", tag="q13")
                nc.vector.tensor_tensor(_as3(q13), _dup2(c4r, nb), _as3(tz), op=OP.mult)
                q24 = sh.tile([128, 2 * nb], BF16, name="q24", tag="q24")
                nc.vector.tensor_tensor(_as3(q24), _dup2(c4i, nb), _swap2(tz, nb),
                                        op=OP.mult)
                r2b = sh.tile([128, nb], BF16, name="r2b", tag="r2b")
                nc.gpsimd.tensor_tensor(r2b, sq3[:, H], sq3[:, I], op=OP.add)
                ht3 = sh.tile([128, 2 * nb], BF16, name="ht3", tag="ht3")
                nc.vector.tensor_tensor(_as3(ht3), _dup2(r2b, nb), _as3(cp3),
                                        op=OP.mult)
                # ---- PE: wave B (W1 -> psB acc, H2 -> psA reuse) ----
                psB = ppool.tile([128, 2 * nb], F32, name="psB", tag="psB", bufs=2)
                for k in range(KT):
                    for hs in (H, I):
                        mm(psB[:, hs], w1[:, k, msl], ch[k][:, hs],
                           start=(k == 0), stop=False)
                psA2 = ppool.tile([128, 2 * nb], F32, name="psA2", tag="psA")
                for k in range(KT):
                    for hs in (H, I):
                        mm(psA2[:, hs], wt[2][:, k, msl], ch[k][:, hs],
                           start=(k == 0), stop=(k == KT - 1))
                sq2 = sh.tile([128, 2 * nb], BF16, name="sq2", tag="sq2")
                nc.scalar.activation(sq2, psA2[:, :], AF.Square, scale=0.5)
                r2a = sh.tile([128, nb], BF16, name="r2a", tag="r2a")
                nc.gpsimd.tensor_tensor(r2a, sq2[:, H], sq2[:, I], op=OP.add)
                return dict(d=d, c=c, m=m, psB=psB, ht3=ht3, q13=q13, q24=q24,
                            r2a=r2a, nxt=nxt, out_t=out_t)
                if d == DEPTH - 1:
                    c0 = c * nb
                    nc.sync.dma_start(out=ore[m * 128:(m + 1) * 128, c0:c0 + nb],
                                      in_=out_t[:, H])
                    nc.sync.dma_start(out=oim[m * 128:(m + 1) * 128, c0:c0 + nb],
                                      in_=out_t[:, I])

            prev = None
            for p in range(n_chunks // 2):
                pair = (2 * p, 2 * p + 1)
                cur_ch = {}
                out_ts = {}
                for c in pair:
                    pc = c & 1
                    cur_ch[c] = []
                    for k in range(KT):
                        t = chpool.tile([128, 2 * nb], BF16, name=f"ch{pc}_{k}",
                                        tag=f"ch{pc}_{k}")
                        nc.sync.dma_start(out=t[:, H],
                                          in_=wre[k * 128:(k + 1) * 128,
                                                  c * nb:(c + 1) * nb])
                        nc.sync.dma_start(out=t[:, I],
                                          in_=wim[k * 128:(k + 1) * 128,
                                                  c * nb:(c + 1) * nb])
                        cur_ch[c].append(t)
                    out_ts[c] = [opool.tile([128, 2 * nb], BF16, name=f"out{pc}_{m}",
                                            tag=f"out{pc}_{m}") for m in range(KT)]
                for d in range(DEPTH):
                    for c in pair:
                        pc = c & 1
                        if d < DEPTH - 1:
                            nxt = [chpool.tile([128, 2 * nb], BF16,
                                               name=f"ch{pc}_{k}", tag=f"ch{pc}_{k}")
                                   for k in range(KT)]
                        else:
                            nxt = None
                        for m in range(KT):
                            prev = emit_step(d, c, m, cur_ch[c], nxt,
                                             out_ts[c][m], prev)
                        cur_ch[c] = nxt if nxt is not None else cur_ch[c]
            emit_identities(prev)
            emit_tanh(prev)
            emit_fin_dve(prev)
    nc.finalize()
    return nc


def host_prep(coupling_matrix, harmonic_1, harmonic_2, harmonic_3, harmonic_5,
              mixing_scale, mixing_bias):
    damping = (0.1 / (1.0 + np.exp(np.linspace(0.0, 3.0, D)))).astype(np.float32)
    w = np.exp(-np.linspace(0.0, 2.0, DEPTH))
    w = (w / w.sum()).astype(np.float32)
    fd = np.stack([np.exp(-damping.astype(np.float64) * float(dd))
                   for dd in range(DEPTH)]).astype(np.float32)      # [7, D]
    wf = (w[:, None] * fd).astype(np.float32)                        # [7, D]
    w1_0 = (coupling_matrix + harmonic_1).astype(np.float32)
    w1_r = (w1_0 + np.eye(D, dtype=np.float32)).astype(np.float32)
    wmat = np.ascontiguousarray(
        np.stack([w1_0, w1_r, harmonic_2, harmonic_3 * H3_FOLD, harmonic_5])
    ).astype(ml_dtypes.bfloat16)
    eye = np.eye(128, dtype=np.float32)
    idm = np.stack([eye, 4.0 * eye, -4.0 * eye]).astype(ml_dtypes.bfloat16)
    consts = np.zeros((D, 16), np.float32)
    consts[:, 0:DEPTH] = fd.T
    consts[:, 7:7 + DEPTH] = wf.T
    consts[:, 6] = EXP_BIAS  # fd_6 never read (no chamber after last depth)
    consts[:, 14] = mixing_scale.astype(np.float32)
    consts[:, 15] = mixing_bias.astype(np.float32)
    return wmat, idm, consts


_NC_CACHE = {}


def _get_nc(n_chunks, nb):
    key = (n_chunks, nb)
    if key not in _NC_CACHE:
        _NC_CACHE[key] = build_program(n_chunks, nb)
    return _NC_CACHE[key]


def kernel(wave_real, wave_imag, coupling_matrix, harmonic_1, harmonic_2,
           harmonic_3, harmonic_5, mixing_scale, mixing_bias):
    wmat, idm, consts = host_prep(coupling_matrix, harmonic_1, harmonic_2,
                                  harmonic_3, harmonic_5, mixing_scale, mixing_bias)
    wreT = np.ascontiguousarray(
        np.asarray(wave_real, np.float32).T).astype(ml_dtypes.bfloat16)  # [D, B]
    wimT = np.ascontiguousarray(
        np.asarray(wave_imag, np.float32).T).astype(ml_dtypes.bfloat16)

    nc = _get_nc(BS // NB, NB)
    in_maps = []
    for c in range(N_CORES):
        sl = slice(c * BS, (c + 1) * BS)
        in_maps.append({
            "wre": np.ascontiguousarray(wreT[:, sl]),
            "wim": np.ascontiguousarray(wimT[:, sl]),
            "wmat": wmat,
            "idm": idm,
            "consts": consts,
        })
    res = bass_utils.run_bass_kernel_spmd(nc, in_maps, core_ids=list(range(N_CORES)))
    out = np.empty((2, B, D), np.float32)
    for c in range(N_CORES):
        sl = slice(c * BS, (c + 1) * BS)
        out[0, sl, :] = res.results[c]["ore"].astype(np.float32).T
        out[1, sl, :] = res.results[c]["oim"].astype(np.float32).T
    return out
